# revision 11
# baseline (speedup 1.0000x reference)
"""Trainium2 Bass/Tile kernel for EnrichedGeometricEmbedding.

Full-input contract: kernel(**inputs) takes the complete tensors, shards the
batch dim across 8 NeuronCores (B=8 -> 1 batch row per core), runs one SPMD
program via run_bass_kernel_spmd, and gathers the full [8, 1024, 32, 384]
output. Memory-bound problem: the 50 MB/core output write (~140 us at
360 GB/s) sets the roofline; everything else is prologue latency before the
first output DMA can start.

Layout: groups are chunked q-r style (g = q*8 + r, q in [0,128) on the
partition dim) so each input DMA descriptor is a 3 KB contiguous run.  Column
order everywhere downstream is c = r*128 + q.  x is transposed to a
(k,d)-partition layout xkdT [96, 1024] (partition 3k+d) via 8 f32r PE
transposes of [128,96] slices.

Prologue critical path (time-to-curv) is minimized:
  - stats: ACT computes the diagonal second moments (Square), Pool (gpsimd)
    the off-diagonal products, DVE only the mean + one batched reduce; the
    covariance is centered from raw moments (U = P - msum*msum/K).
  - smallest eigenvalue: trig-free.  lam_min = tr/3 + p * f(r) with
    f(r) = 2cos(acos(r)/3 + 2pi/3) evaluated as a degree-12 polynomial
    (max curv abs err 3.4e-3, tolerance ~5e-2), and rsqrt(p2s) via the
    int32 bit-trick + 2 Newton steps - the whole chain lives on DVE, so the
    ACT engine keeps a single function table (exp_and_others) for the whole
    kernel: zero LoadActFuncSet swaps after the initial load.
  - lap rows: y = x - mean subtracted in natural layout on Pool, transposed
    on PE, and the PSUM->SBUF evacuation IS the Abs (ACT reads PSUM).
  - the 2.3 MB one-hot broadcast matrix ebig is built on-chip (memset + 32
    tiny SBUF DMAs from a [3,128] seed) instead of being DMA'd from DRAM.

Main loop (4 phases x 8 k x 2 halves of 512 points) is unchanged in spirit
from the tuned baseline and runs DMA-bound: xb[128,512] = ebig_k^T @ xkdT
broadcasts x to the 128 rbf rows; rbf = Exp(-2*Square(x - c)) on ACT with
per-partition constants; per 128-point tile one K=128 (whi) + one K=6 (wlo)
accumulating f32r matmul into a [128,512] PSUM tile; PSUM->SBUF copies
alternate DVE/ACT; one HWDGE DMA per 512 points scatters [128, 4, 384] rows
to DRAM (1536 B descriptors).  The ones row folds the projection bias into
the matmul.
"""

import numpy as np

B, S, K, D = 8, 1024, 32, 3
F = 43                      # FEAT_DIM
OUT = 384
G = S                       # groups per core
P = S * K                   # points per core (32768)
R = 8                       # groups-per-partition (g = q*8 + r)
TOTAL = F * D + 1 + D       # 133
HK = K // 4                 # k per phase

_prog_cache = {}


def _poly_coeffs():
    """Power-basis coefficients of the deg-12 fit to the lambda_min factor."""
    rr = np.linspace(-1.0, 1.0, 200001)
    f = 2.0 * np.cos(np.arccos(rr) / 3.0 + 2.0 * np.pi / 3.0)
    cheb = np.polynomial.chebyshev.Chebyshev.fit(rr, f, 12)
    return np.polynomial.chebyshev.cheb2poly(cheb.convert().coef)


def _build_program():
    import concourse.bacc as bacc
    import concourse.mybir as mybir
    from concourse.tile import TileContext

    DT = mybir.dt.float32
    DTI = mybir.dt.int32
    DTR = mybir.dt.float32r
    Act = mybir.ActivationFunctionType
    Op = mybir.AluOpType
    X = mybir.AxisListType.X

    C = np.linspace(-1.0, 1.0, F + 2, dtype=np.float64)[1:-1]
    C42 = float(C[F - 1])
    A = _poly_coeffs()          # a[0..12]
    MAGIC = 0x5F3759DF

    nc = bacc.Bacc("TRN2", target_bir_lowering=False, debug=False, num_devices=8)
    xyz_d = nc.dram_tensor("xyz", [P, D], DTR, kind="ExternalInput").ap()
    nbr_d = nc.dram_tensor("nbr", [P, D], DT, kind="ExternalInput").ap()
    # blob_a = ident | E_base | negc   (needed early: transposes + ebig build)
    NBA = 128 + 128 + 1
    # blob_b = whi | wlo6              (needed at the first output matmul)
    NBB = OUT + OUT
    bloba_d = nc.dram_tensor("blob_a", [128, NBA], DTR, kind="ExternalInput").ap()
    blobb_d = nc.dram_tensor("blob_b", [128, NBB], DTR, kind="ExternalInput").ap()
    out_d = nc.dram_tensor("out", [P, OUT], DT, kind="ExternalOutput").ap()

    def view_ti(t24, width, i):
        # column i of a [128, r*width] tile laid out (r, i) -> [128, r]
        return t24.rearrange("p (t i) -> p i t", i=width)[:, i : i + 1, :].squeeze(1)

    with TileContext(nc) as tc:
        with (
            tc.tile_pool(name="const", bufs=1) as constp,
            tc.tile_pool(name="stats", bufs=1) as statp,
            tc.tile_pool(name="gwork", bufs=8) as gwp,
            tc.tile_pool(name="flopool", bufs=1) as flop,
            tc.tile_pool(name="main", bufs=6) as mainp,
        ):
            poolA = tc.alloc_tile_pool(name="tpsum", bufs=2, space="PSUM")
            poolB = tc.alloc_tile_pool(name="cpsum", bufs=1, space="PSUM")
            xbp = tc.alloc_tile_pool(name="xbpsum", bufs=2, space="PSUM")
            outp = tc.alloc_tile_pool(name="outpsum", bufs=3, space="PSUM")

            # ---- constants built on-chip ----
            ebig = constp.tile([96, K * 128], DTR)
            nc.vector.memset(ebig.bitcast(DT)[:], 0.0)
            ones_t = gwp.tile([128, HK * G // 128], DT, tag="ones", bufs=1)
            nc.vector.memset(ones_t[:], 1.0)

            # ---- input DMAs (sync queue, critical-path order) ----
            n_all = gwp.tile([128, R * K * D], DT, tag="nall", bufs=1)
            nc.sync.dma_start(
                n_all[:], nbr_d.rearrange("(q w) d -> q (w d)", q=128)
            )
            bloba = constp.tile([128, NBA], DTR)
            nc.sync.dma_start(bloba[:], bloba_d[:])
            identR = bloba[:, 0:128]
            ident = identR.bitcast(DT)
            ebase = bloba[0:D, 128:256]
            negc = bloba[:, 256:257].bitcast(DT)
            x_all = gwp.tile([128, R * K * D], DTR, tag="xall", bufs=1)
            nc.sync.dma_start(
                x_all[:], xyz_d.rearrange("(q w) d -> q (w d)", q=128)
            )
            blobb = constp.tile([128, NBB], DTR)
            nc.sync.dma_start(blobb[:], blobb_d[:])
            whi = blobb[:, 0:OUT]
            wlo = blobb[0:6, OUT : 2 * OUT]
            # ebig one-hot blocks: ebig[3k+d, 128k + r] = 1 iff r//43 == d
            for k in range(K):
                nc.sync.dma_start(
                    ebig[3 * k : 3 * k + 3, 128 * k : 128 * (k + 1)], ebase
                )

            # ---- stats: raw moments, mean; products split ACT/Pool ----
            n_v = n_all.rearrange("q (r k d) -> q r d k", k=K, d=D)
            x_v = x_all.bitcast(DT).rearrange("q (r k d) -> q r d k", k=K, d=D)
            prod = gwp.tile([128, R * 6 * K], DT, tag="prod", bufs=1)
            prod_v = prod.rearrange("q (r j k) -> q r j k", j=6, k=K)
            msum = statp.tile([128, R * D], DT)
            msum_v = msum.rearrange("q (r d) -> q r d", d=D)
            # DVE: mean (raw sums; scaling folded into consumers)
            nc.vector.tensor_reduce(msum_v, n_v, axis=X, op=Op.add)
            # ACT: diagonal second moments
            nc.scalar.activation(prod_v[:, :, 0:3, :], n_v, Act.Square)
            # Pool: off-diagonal products
            nc.gpsimd.tensor_mul(
                prod_v[:, :, 3:5, :], n_v[:, :, 0:2, :], n_v[:, :, 1:3, :]
            )
            nc.gpsimd.tensor_mul(
                prod_v[:, :, 5:6, :], n_v[:, :, 0:1, :], n_v[:, :, 2:3, :]
            )
            # Pool: y = x - msum/K  (lap pre-subtraction, natural layout)
            y_all = gwp.tile([128, R * K * D], DTR, tag="yall", bufs=1)
            y_v = y_all.bitcast(DT).rearrange("q (r k d) -> q r d k", k=K, d=D)
            mneg = statp.tile([128, R * D], DT)
            nc.gpsimd.tensor_scalar_mul(mneg[:], msum[:], -1.0 / K)
            y_vr = y_all.rearrange("q (r k d) -> q r d k", k=K, d=D)
            nc.gpsimd.tensor_add(
                y_vr,
                x_v,
                mneg.rearrange("q (r d) -> q r d", d=D)
                .unsqueeze(3)
                .broadcast_to([128, R, D, K]),
            )

            # ---- PE transposes: x then y, 4 r-slices per PSUM bank ----
            xkdT = statp.tile([96, G], DTR)
            lapT = statp.tile([96, G], DT)
            for half in range(2):
                tp = poolA.tile([96, 512], DT, tag="tp")
                for rr in range(4):
                    r = half * 4 + rr
                    nc.tensor.transpose(
                        tp[:, 128 * rr : 128 * (rr + 1)].bitcast(DTR),
                        x_all[:, 96 * r : 96 * (r + 1)],
                        identR,
                    )
                nc.scalar.copy(xkdT[:, 512 * half : 512 * (half + 1)], tp[:])
            for half in range(2):
                tp = poolA.tile([96, 512], DT, tag="tp")
                for rr in range(4):
                    r = half * 4 + rr
                    nc.tensor.transpose(
                        tp[:, 128 * rr : 128 * (rr + 1)].bitcast(DTR),
                        y_all[:, 96 * r : 96 * (r + 1)],
                        identR,
                    )
                # PSUM evacuation doubles as the Abs
                nc.scalar.activation(
                    lapT[:, 512 * half : 512 * (half + 1)], tp[:], Act.Abs
                )

            # ---- g42 row: gaussian of the d=2 plane (pure ACT) ----
            c42b = constp.tile([96, 1], DT)
            nc.vector.memset(c42b[:], -C42)
            g42s = statp.tile([96, G], DT)
            nc.scalar.activation(g42s[:], xkdT.bitcast(DT)[:], Act.Square, bias=c42b[:])
            g42f = statp.tile([96, G], DT)
            nc.scalar.activation(g42f[:], g42s[:], Act.Exp, scale=-2.0)

            # ---- covariance from raw moments ----
            U = statp.tile([128, R * 6], DT)
            nc.vector.tensor_reduce(
                U[:], prod.rearrange("q (x k) -> q x k", k=K), axis=X, op=Op.add
            )
            mm = statp.tile([128, R * 6], DT)
            mm_v = mm.rearrange("q (r j) -> q r j", j=6)
            nc.vector.tensor_mul(mm_v[:, :, 0:3], msum_v, msum_v)
            nc.vector.tensor_mul(mm_v[:, :, 3:5], msum_v[:, :, 0:2], msum_v[:, :, 1:3])
            nc.vector.tensor_mul(mm_v[:, :, 5:6], msum_v[:, :, 0:1], msum_v[:, :, 2:3])
            nc.vector.scalar_tensor_tensor(
                U[:], mm[:], -1.0 / K, U[:], op0=Op.mult, op1=Op.add
            )

            # ---- eigen chain, all DVE ----
            U_rj = U.rearrange("q (r j) -> q r j", j=6)
            tr_t = statp.tile([128, R], DT)
            nc.vector.tensor_reduce(tr_t[:], U_rj[:, :, 0:3], axis=X, op=Op.add)
            dd = statp.tile([128, R * 3], DT)
            dd_v = dd.rearrange("q (r i) -> q r i", i=3)
            nc.vector.scalar_tensor_tensor(
                dd_v,
                tr_t[:].unsqueeze(2).broadcast_to([128, R, 3]),
                -1.0 / 3.0,
                U_rj[:, :, 0:3],
                op0=Op.mult,
                op1=Op.add,
            )
            dd2 = statp.tile([128, R * 3], DT)
            nc.vector.tensor_mul(dd2[:], dd[:], dd[:])
            sdd = statp.tile([128, R], DT)
            nc.vector.tensor_reduce(
                sdd[:], dd2.rearrange("q (r i) -> q r i", i=3), axis=X, op=Op.add
            )
            sqo = statp.tile([128, R * 3], DT)
            sqo_v = sqo.rearrange("q (r i) -> q r i", i=3)
            nc.vector.tensor_mul(sqo_v, U_rj[:, :, 3:6], U_rj[:, :, 3:6])
            s2 = statp.tile([128, R], DT)
            nc.vector.tensor_reduce(s2[:], sqo_v, axis=X, op=Op.add)
            p2s = statp.tile([128, R], DT)
            nc.vector.scalar_tensor_tensor(
                p2s[:], s2[:], 2.0, sdd[:], op0=Op.mult, op1=Op.add
            )
            nc.vector.tensor_scalar(
                p2s[:], p2s[:], 1.0 / 6.0, 1e-24, op0=Op.mult, op1=Op.add
            )
            # rsqrt(p2s): int32 bit-trick + 2 Newton steps
            ish = statp.tile([128, R], DT)
            nc.vector.tensor_scalar(
                ish.bitcast(DTI)[:], p2s.bitcast(DTI)[:], 1, None,
                op0=Op.arith_shift_right,
            )
            rsq = statp.tile([128, R], DT)
            nc.vector.tensor_scalar(
                rsq.bitcast(DTI)[:], ish.bitcast(DTI)[:], -1, MAGIC,
                op0=Op.mult, op1=Op.add,
            )
            hlf = statp.tile([128, R], DT)
            nc.vector.tensor_scalar_mul(hlf[:], p2s[:], 0.5)
            nt = statp.tile([128, R], DT)
            for _ in range(2):
                nc.vector.tensor_mul(nt[:], rsq[:], rsq[:])
                nc.vector.tensor_mul(nt[:], nt[:], hlf[:])
                nc.vector.tensor_scalar(
                    nt[:], nt[:], -1.0, 1.5, op0=Op.mult, op1=Op.add
                )
                nc.vector.tensor_mul(rsq[:], rsq[:], nt[:])
            # det of the shifted matrix
            d0 = view_ti(dd, 3, 0)
            d1 = view_ti(dd, 3, 1)
            d2 = view_ti(dd, 3, 2)
            o01 = view_ti(U, 6, 3)
            o12 = view_ti(U, 6, 4)
            o02 = view_ti(U, 6, 5)
            q0 = view_ti(sqo, 3, 0)
            q1 = view_ti(sqo, 3, 1)
            q2 = view_ti(sqo, 3, 2)
            det = statp.tile([128, R], DT)
            scr = statp.tile([128, R], DT)
            nc.vector.tensor_mul(det[:], d1, d2)
            nc.vector.tensor_mul(det[:], det[:], d0)
            nc.vector.tensor_mul(scr[:], o01, o12)
            nc.vector.scalar_tensor_tensor(
                scr[:], scr[:], 2.0, o02, op0=Op.mult, op1=Op.mult
            )
            nc.vector.tensor_add(det[:], det[:], scr[:])
            nc.vector.tensor_mul(scr[:], d0, q1)
            nc.vector.tensor_sub(det[:], det[:], scr[:])
            nc.vector.tensor_mul(scr[:], d1, q2)
            nc.vector.tensor_sub(det[:], det[:], scr[:])
            nc.vector.tensor_mul(scr[:], d2, q0)
            nc.vector.tensor_sub(det[:], det[:], scr[:])
            # r = det/2 * rsq^3, clamped
            rec3 = statp.tile([128, R], DT)
            nc.vector.tensor_mul(rec3[:], rsq[:], rsq[:])
            nc.vector.tensor_mul(rec3[:], rec3[:], rsq[:])
            r_t = statp.tile([128, R], DT)
            nc.vector.scalar_tensor_tensor(
                r_t[:], det[:], 0.5, rec3[:], op0=Op.mult, op1=Op.mult
            )
            nc.vector.tensor_scalar(
                r_t[:], r_t[:], 0.999999, -0.999999, op0=Op.min, op1=Op.max
            )
            # f(r): degree-12 polynomial, stt-form Horner
            acc = statp.tile([128, R], DT)
            nc.vector.tensor_scalar(
                acc[:], r_t[:], float(A[12]), float(A[11]), op0=Op.mult, op1=Op.add
            )
            for c in [0.0] + [float(A[k]) for k in range(10, 0, -1)]:
                nc.vector.scalar_tensor_tensor(
                    acc[:], acc[:], c, r_t[:], op0=Op.add, op1=Op.mult
                )
            nc.vector.tensor_scalar_add(acc[:], acc[:], float(A[0]))
            # lam = tr/3 + p * f;  curv = lam / (tr + (K-1)*1e-6)
            pt = statp.tile([128, R], DT)
            nc.vector.tensor_mul(pt[:], p2s[:], rsq[:])
            nc.vector.tensor_mul(pt[:], pt[:], acc[:])
            lam = statp.tile([128, R], DT)
            nc.vector.scalar_tensor_tensor(
                lam[:], tr_t[:], 1.0 / 3.0, pt[:], op0=Op.mult, op1=Op.add
            )
            den = statp.tile([128, R], DT)
            nc.vector.tensor_scalar_add(den[:], tr_t[:], (K - 1) * 1e-6)
            dr = statp.tile([128, R], DT)
            nc.vector.reciprocal(dr[:], den[:])
            curv_all = statp.tile([128, R], DT)
            nc.vector.tensor_mul(curv_all[:], lam[:], dr[:])

            # curv to column order: [128q, 8r] -> [8r, 128q]
            cps = poolB.tile([8, 128], DT, tag="cps")
            nc.tensor.transpose(cps[:], curv_all[:], ident)
            ctv = statp.tile([8, 128], DT)
            nc.vector.tensor_copy(ctv[:], cps[:])
            curv_c = statp.tile([1, G], DT)
            nc.sync.dma_start(
                curv_c.rearrange("o (r q) -> o r q", q=128), ctv[:]
            )

            # strided-partition views for the flo rows (partition 3k+d)
            g42_kd = g42f.rearrange("(k d) c -> d k c", d=D)
            lap_kd = lapT.rearrange("(k d) c -> d k c", d=D)

            # ---- main loop: 4 phases x 8 k x 2 halves ----
            for phase in range(4):
                k0 = phase * HK
                flo = flop.tile([6, HK * G], DTR, tag="flo", bufs=2)
                nc.sync.dma_start(
                    flo[0:1, :].rearrange("o (k c) -> o k c", c=G),
                    g42_kd[2:3, k0 : k0 + HK, :].squeeze(0).bitcast(DTR),
                )
                nc.sync.dma_start(
                    flo[1:2, :].rearrange("o (k c) -> o k c", c=G),
                    curv_c.bitcast(DTR).unsqueeze(1).broadcast_to([1, HK, G]),
                )
                for d in range(D):
                    nc.sync.dma_start(
                        flo[2 + d : 3 + d, :].rearrange("o (k c) -> o k c", c=G),
                        lap_kd[d : d + 1, k0 : k0 + HK, :].squeeze(0).bitcast(DTR),
                    )
                nc.sync.dma_start(
                    flo[5:6, :].rearrange("o (a b) -> o a b", b=HK * G // 128),
                    ones_t.bitcast(DTR),
                )
                for k in range(k0, k0 + HK):
                    for half in range(2):
                        csl = slice(half * 512, (half + 1) * 512)
                        xb = xbp.tile([128, 512], DT, tag="xb")
                        nc.tensor.matmul(
                            xb[:],
                            ebig[:, k * 128 : (k + 1) * 128],
                            xkdT[:, csl],
                            start=True,
                            stop=True,
                        )
                        # rbf = Exp(-2 * (x - c)^2), c per rbf row
                        t2 = mainp.tile([128, 512], DT, tag="t2")
                        nc.scalar.activation(t2[:], xb[:], Act.Square, bias=negc)
                        fhi = mainp.tile([128, 512], DTR, tag="fhi")
                        nc.scalar.activation(fhi[:], t2[:], Act.Exp, scale=-2.0)
                        so = mainp.tile([128, 4 * OUT], DT, tag="so", bufs=6)
                        so_v = so.rearrange("p (c x) -> p c x", x=OUT)
                        for j in range(4):
                            ps = outp.tile([128, 512], DT, tag="ps")
                            nc.tensor.matmul(
                                ps[:, 0:OUT],
                                fhi[:, j * 128 : (j + 1) * 128],
                                whi,
                                start=True,
                                stop=False,
                            )
                            lo = (k - k0) * G + half * 512 + j * 128
                            nc.tensor.matmul(
                                ps[:, 0:OUT],
                                flo[0:6, lo : lo + 128],
                                wlo,
                                start=False,
                                stop=True,
                            )
                            cnt = ((k * 2 + half) * 4 + j)
                            dst = so_v[:, j : j + 1, :]
                            if cnt % 2 == 1:
                                nc.scalar.copy(dst, ps[:, 0:OUT].unsqueeze(1))
                            else:
                                nc.vector.tensor_copy(dst, ps[:, 0:OUT].unsqueeze(1))
                        # rows p = g*32 + k = (q*8 + (half*4 + c))*32 + k
                        out_ap = out_d.rearrange(
                            "(q r k) x -> k q r x", r=R, k=K
                        )[k : k + 1, :, half * 4 : (half + 1) * 4, :].squeeze(0)
                        nc.sync.dma_start(
                            out_ap, so.rearrange("p (c x) -> p c x", x=OUT)
                        )
            outp.release()
            xbp.release()
            poolB.release()
            poolA.release()

    nc.compile()
    return nc


def _get_program():
    if "nc" not in _prog_cache:
        _prog_cache["nc"] = _build_program()
    return _prog_cache["nc"]


def kernel(xyz, neighbor_xyz, projection_weight, projection_bias):
    from concourse.bass_utils import run_bass_kernel_spmd

    nc = _get_program()

    w = np.ascontiguousarray(projection_weight, dtype=np.float32)
    bias = np.ascontiguousarray(projection_bias, dtype=np.float32)
    whi = np.ascontiguousarray(w[:128])
    wlo6 = np.concatenate([w[128:TOTAL], bias[None, :]], axis=0).astype(np.float32)

    ident = np.eye(128, dtype=np.float32)
    ebase = np.zeros((D, 128), dtype=np.float32)
    for r in range(128):
        ebase[r // F, r] = 1.0
    c = np.linspace(-1.0, 1.0, F + 2, dtype=np.float32)[1:-1]
    negc = (-c[np.arange(128) % F]).reshape(128, 1).astype(np.float32)

    NBA = 128 + 128 + 1
    blob_a = np.zeros((128, NBA), dtype=np.float32)
    blob_a[:, 0:128] = ident
    blob_a[0:D, 128:256] = ebase
    blob_a[:, 256:257] = negc
    blob_b = np.zeros((128, OUT + OUT), dtype=np.float32)
    blob_b[:, 0:OUT] = whi
    blob_b[0:6, OUT : 2 * OUT] = wlo6

    xyz = np.ascontiguousarray(xyz, dtype=np.float32)
    nbr = np.ascontiguousarray(neighbor_xyz, dtype=np.float32)
    in_maps = []
    for core in range(B):
        in_maps.append(
            {
                "xyz": xyz[core].reshape(P, D),
                "nbr": nbr[core].reshape(P, D),
                "blob_a": blob_a,
                "blob_b": blob_b,
            }
        )
    res = run_bass_kernel_spmd(nc, in_maps, list(range(B)))
    out = np.empty((B, S, K, OUT), dtype=np.float32)
    for i in range(B):
        # rows are p = (q*8 + r)*32 + k in natural order already
        out[i] = res.results[i]["out"].reshape(S, K, OUT)
    return out


# revision 15
# speedup vs baseline: 1.1491x; 1.1491x over previous
"""Trainium2 Bass/Tile kernel for EnrichedGeometricEmbedding.

Full-input contract: kernel(**inputs) takes the complete tensors, shards the
batch dim across 8 NeuronCores (B=8 -> 1 batch row per core), runs one SPMD
program via run_bass_kernel_spmd, and gathers the full [8, 1024, 32, 384]
output. Memory-bound problem: the 50 MB/core output write (~140 us at
360 GB/s) sets the roofline; everything else is prologue latency before the
first output DMA can start.

Layout: groups are chunked q-r style (g = q*8 + r, q in [0,128) on the
partition dim) so each input DMA descriptor is a 3 KB contiguous run.  Column
order everywhere downstream is c = r*128 + q.  x is transposed to a
(k,d)-partition layout xkdT [96, 1024] (partition 3k+d) via 8 f32r PE
transposes of [128,96] slices.

Prologue critical path (time-to-curv) is minimized:
  - stats: ACT computes the diagonal second moments (Square), Pool (gpsimd)
    the off-diagonal products, DVE only the mean + one batched reduce; the
    covariance is centered from raw moments (U = P - msum*msum/K).
  - smallest eigenvalue: trig-free.  lam_min = tr/3 + p * f(r) with
    f(r) = 2cos(acos(r)/3 + 2pi/3) evaluated as a degree-12 polynomial
    (max curv abs err 3.4e-3, tolerance ~5e-2), and rsqrt(p2s) via the
    int32 bit-trick + 2 Newton steps - the whole chain lives on DVE, so the
    ACT engine keeps a single function table (exp_and_others) for the whole
    kernel: zero LoadActFuncSet swaps after the initial load.
  - lap rows: y = x - mean subtracted in natural layout on Pool, transposed
    on PE, and the PSUM->SBUF evacuation IS the Abs (ACT reads PSUM).
  - the 2.3 MB one-hot broadcast matrix ebig is built on-chip (memset + 32
    tiny SBUF DMAs from a [3,128] seed) instead of being DMA'd from DRAM.

Main loop (4 phases x 8 k x 2 halves of 512 points) is unchanged in spirit
from the tuned baseline and runs DMA-bound: xb[128,512] = ebig_k^T @ xkdT
broadcasts x to the 128 rbf rows; rbf = Exp(-2*Square(x - c)) on ACT with
per-partition constants; per 128-point tile one K=128 (whi) + one K=6 (wlo)
accumulating f32r matmul into a [128,512] PSUM tile; PSUM->SBUF copies
alternate DVE/ACT; one HWDGE DMA per 512 points scatters [128, 4, 384] rows
to DRAM (1536 B descriptors).  The ones row folds the projection bias into
the matmul.
"""

import numpy as np

B, S, K, D = 8, 1024, 32, 3
F = 43                      # FEAT_DIM
OUT = 384
G = S                       # groups per core
P = S * K                   # points per core (32768)
R = 8                       # groups-per-partition (g = q*8 + r)
TOTAL = F * D + 1 + D       # 133
HK = K // 4                 # k per phase

_prog_cache = {}


def _poly_coeffs():
    """Power-basis coefficients of the deg-12 fit to the lambda_min factor."""
    rr = np.linspace(-1.0, 1.0, 200001)
    f = 2.0 * np.cos(np.arccos(rr) / 3.0 + 2.0 * np.pi / 3.0)
    cheb = np.polynomial.chebyshev.Chebyshev.fit(rr, f, 10)
    return np.polynomial.chebyshev.cheb2poly(cheb.convert().coef)


def _build_program():
    import concourse.bacc as bacc
    import concourse.mybir as mybir
    from concourse.tile import TileContext

    DT = mybir.dt.float32
    DTI = mybir.dt.int32
    DTR = mybir.dt.float32r
    Act = mybir.ActivationFunctionType
    Op = mybir.AluOpType
    X = mybir.AxisListType.X

    C = np.linspace(-1.0, 1.0, F + 2, dtype=np.float64)[1:-1]
    C42 = float(C[F - 1])
    A = _poly_coeffs()          # a[0..12]
    MAGIC = 0x5F3759DF

    nc = bacc.Bacc("TRN2", target_bir_lowering=False, debug=False, num_devices=8)
    xyz_d = nc.dram_tensor("xyz", [P, D], DTR, kind="ExternalInput").ap()
    nbr_d = nc.dram_tensor("nbr", [P, D], DT, kind="ExternalInput").ap()
    # blob_a = ident | E_base | negc   (needed early: transposes + ebig build)
    NBA = 128 + 128 + 1
    # blob_b = whi | wlo6              (needed at the first output matmul)
    NBB = OUT + OUT
    bloba_d = nc.dram_tensor("blob_a", [128, NBA], DTR, kind="ExternalInput").ap()
    blobb_d = nc.dram_tensor("blob_b", [128, NBB], DTR, kind="ExternalInput").ap()
    out_d = nc.dram_tensor("out", [P, OUT], DT, kind="ExternalOutput").ap()

    def view_ti(t24, width, i):
        # column i of a [128, r*width] tile laid out (r, i) -> [128, r]
        return t24.rearrange("p (t i) -> p i t", i=width)[:, i : i + 1, :].squeeze(1)

    with TileContext(nc) as tc:
        with (
            tc.tile_pool(name="const", bufs=1) as constp,
            tc.tile_pool(name="stats", bufs=1) as statp,
            tc.tile_pool(name="gwork", bufs=8) as gwp,
            tc.tile_pool(name="flopool", bufs=1) as flop,
            tc.tile_pool(name="main", bufs=6) as mainp,
        ):
            xbp = tc.alloc_tile_pool(name="xbpsum", bufs=2, space="PSUM")
            poolA = tc.alloc_tile_pool(name="tpsum", bufs=2, space="PSUM")
            poolB = tc.alloc_tile_pool(name="cpsum", bufs=1, space="PSUM")

            # ---- constants built on-chip ----
            ebig = constp.tile([128, K * 128], DTR)
            nc.vector.memset(ebig.bitcast(DT)[:], 0.0)
            ones_t = gwp.tile([128, HK * G // 128], DT, tag="ones", bufs=1)
            nc.vector.memset(ones_t[:], 1.0)

            # ---- input DMAs (sync queue, critical-path order) ----
            n_all = gwp.tile([128, R * K * D], DT, tag="nall", bufs=1)
            nc.sync.dma_start(
                n_all[:], nbr_d.rearrange("(q w) d -> q (w d)", q=128)
            )
            bloba = constp.tile([128, NBA], DTR)
            nc.sync.dma_start(bloba[:], bloba_d[:])
            identR = bloba[:, 0:128]
            ident = identR.bitcast(DT)
            ebase = bloba[0:D, 128:256]
            negc = bloba[:, 256:257].bitcast(DT)
            x_all = gwp.tile([128, R * K * D], DTR, tag="xall", bufs=1)
            nc.sync.dma_start(
                x_all[:], xyz_d.rearrange("(q w) d -> q (w d)", q=128)
            )
            blobb = constp.tile([128, NBB], DTR)
            nc.sync.dma_start(blobb[:], blobb_d[:])
            whi = blobb[:, 0:OUT]
            wlo = blobb[0:6, OUT : 2 * OUT]
            # ebig one-hot diagonal: ebig[3k+d, 128k + r] = 1 iff r//43 == d.
            # Built by log-doubling: seed block 0, then copy the leading
            # [3*2^i, 128*2^i] diagonal square to the next diagonal position.
            nc.sync.dma_start(ebig[0:3, 0:128], ebase)
            for i in range(5):
                w = 1 << i
                nc.sync.dma_start(
                    ebig[3 * w : 6 * w, 128 * w : 256 * w],
                    ebig[0 : 3 * w, 0 : 128 * w],
                )

            # ---- stats: raw moments, mean; products split ACT/Pool ----
            n_v = n_all.rearrange("q (r k d) -> q r d k", k=K, d=D)
            x_v = x_all.bitcast(DT).rearrange("q (r k d) -> q r d k", k=K, d=D)
            prod = gwp.tile([128, R * 6 * K], DT, tag="prod", bufs=1)
            prod_v = prod.rearrange("q (r j k) -> q r j k", j=6, k=K)
            msum = statp.tile([128, R * D], DT)
            msum_v = msum.rearrange("q (r d) -> q r d", d=D)
            # DVE: mean (raw sums; scaling folded into consumers)
            nc.vector.tensor_reduce(msum_v, n_v, axis=X, op=Op.add)
            # ACT: diagonal second moments
            nc.scalar.activation(prod_v[:, :, 0:3, :], n_v, Act.Square)
            # Pool: off-diagonal products
            nc.gpsimd.tensor_mul(
                prod_v[:, :, 3:5, :], n_v[:, :, 0:2, :], n_v[:, :, 1:3, :]
            )
            nc.vector.tensor_mul(
                prod_v[:, :, 5:6, :], n_v[:, :, 0:1, :], n_v[:, :, 2:3, :]
            )
            # Pool: y = x - msum/K  (lap pre-subtraction, natural layout)
            y_all = gwp.tile([128, R * K * D], DTR, tag="yall", bufs=1)
            y_v = y_all.bitcast(DT).rearrange("q (r k d) -> q r d k", k=K, d=D)
            mneg = statp.tile([128, R * D], DT)
            nc.gpsimd.tensor_scalar_mul(mneg[:], msum[:], -1.0 / K)
            y_vr = y_all.rearrange("q (r k d) -> q r d k", k=K, d=D)
            nc.gpsimd.tensor_add(
                y_vr,
                x_v,
                mneg.rearrange("q (r d) -> q r d", d=D)
                .unsqueeze(3)
                .broadcast_to([128, R, D, K]),
            )

            # ---- PE transposes: x then y, 4 r-slices per PSUM bank ----
            xkdT = statp.tile([96, G], DTR)
            lapT = statp.tile([96, G], DT)
            for half in range(2):
                tp = poolA.tile([96, 512], DT, tag="tp")
                for rr in range(4):
                    r = half * 4 + rr
                    nc.tensor.transpose(
                        tp[:, 128 * rr : 128 * (rr + 1)].bitcast(DTR),
                        x_all[:, 96 * r : 96 * (r + 1)],
                        identR,
                    )
                nc.scalar.copy(xkdT[:, 512 * half : 512 * (half + 1)], tp[:])
            for half in range(2):
                tp = poolA.tile([96, 512], DT, tag="tp")
                for rr in range(4):
                    r = half * 4 + rr
                    nc.tensor.transpose(
                        tp[:, 128 * rr : 128 * (rr + 1)].bitcast(DTR),
                        y_all[:, 96 * r : 96 * (r + 1)],
                        identR,
                    )
                # PSUM evacuation doubles as the Abs
                nc.scalar.activation(
                    lapT[:, 512 * half : 512 * (half + 1)], tp[:], Act.Abs
                )

            # ---- g42 row: gaussian of the d=2 plane (pure ACT) ----
            c42b = constp.tile([96, 1], DT)
            nc.vector.memset(c42b[:], -C42)
            g42s = statp.tile([96, G], DT)
            nc.scalar.activation(g42s[:], xkdT.bitcast(DT)[:], Act.Square, bias=c42b[:])
            g42f = statp.tile([96, G], DT)
            nc.scalar.activation(g42f[:], g42s[:], Act.Exp, scale=-2.0)

            # ---- covariance from raw moments ----
            U = statp.tile([128, R * 6], DT)
            nc.vector.tensor_reduce(
                U[:], prod.rearrange("q (x k) -> q x k", k=K), axis=X, op=Op.add
            )
            mm = statp.tile([128, R * 6], DT)
            mm_v = mm.rearrange("q (r j) -> q r j", j=6)
            nc.vector.tensor_mul(mm_v[:, :, 0:3], msum_v, msum_v)
            nc.vector.tensor_mul(mm_v[:, :, 3:5], msum_v[:, :, 0:2], msum_v[:, :, 1:3])
            nc.vector.tensor_mul(mm_v[:, :, 5:6], msum_v[:, :, 0:1], msum_v[:, :, 2:3])
            nc.vector.scalar_tensor_tensor(
                U[:], mm[:], -1.0 / K, U[:], op0=Op.mult, op1=Op.add
            )

            # ---- eigen chain, all DVE ----
            U_rj = U.rearrange("q (r j) -> q r j", j=6)
            tr_t = statp.tile([128, R], DT)
            nc.vector.tensor_reduce(tr_t[:], U_rj[:, :, 0:3], axis=X, op=Op.add)
            dd = statp.tile([128, R * 3], DT)
            dd_v = dd.rearrange("q (r i) -> q r i", i=3)
            nc.vector.scalar_tensor_tensor(
                dd_v,
                tr_t[:].unsqueeze(2).broadcast_to([128, R, 3]),
                -1.0 / 3.0,
                U_rj[:, :, 0:3],
                op0=Op.mult,
                op1=Op.add,
            )
            dd2 = statp.tile([128, R * 3], DT)
            nc.vector.tensor_mul(dd2[:], dd[:], dd[:])
            sdd = statp.tile([128, R], DT)
            nc.vector.tensor_reduce(
                sdd[:], dd2.rearrange("q (r i) -> q r i", i=3), axis=X, op=Op.add
            )
            sqo = statp.tile([128, R * 3], DT)
            sqo_v = sqo.rearrange("q (r i) -> q r i", i=3)
            nc.vector.tensor_mul(sqo_v, U_rj[:, :, 3:6], U_rj[:, :, 3:6])
            s2 = statp.tile([128, R], DT)
            nc.vector.tensor_reduce(s2[:], sqo_v, axis=X, op=Op.add)
            p2s = statp.tile([128, R], DT)
            nc.vector.scalar_tensor_tensor(
                p2s[:], s2[:], 2.0, sdd[:], op0=Op.mult, op1=Op.add
            )
            nc.vector.tensor_scalar(
                p2s[:], p2s[:], 1.0 / 6.0, 1e-24, op0=Op.mult, op1=Op.add
            )
            # rsqrt(p2s): int32 bit-trick + 2 Newton steps
            ish = statp.tile([128, R], DT)
            nc.vector.tensor_scalar(
                ish.bitcast(DTI)[:], p2s.bitcast(DTI)[:], 1, None,
                op0=Op.arith_shift_right,
            )
            rsq = statp.tile([128, R], DT)
            nc.vector.tensor_scalar(
                rsq.bitcast(DTI)[:], ish.bitcast(DTI)[:], -1, MAGIC,
                op0=Op.mult, op1=Op.add,
            )
            hlf = statp.tile([128, R], DT)
            nc.vector.tensor_scalar_mul(hlf[:], p2s[:], 0.5)
            nt = statp.tile([128, R], DT)
            for _ in range(1):
                nc.vector.tensor_mul(nt[:], rsq[:], rsq[:])
                nc.vector.tensor_mul(nt[:], nt[:], hlf[:])
                nc.vector.tensor_scalar(
                    nt[:], nt[:], -1.0, 1.5, op0=Op.mult, op1=Op.add
                )
                nc.vector.tensor_mul(rsq[:], rsq[:], nt[:])
            # det of the shifted matrix
            d0 = view_ti(dd, 3, 0)
            d1 = view_ti(dd, 3, 1)
            d2 = view_ti(dd, 3, 2)
            o01 = view_ti(U, 6, 3)
            o12 = view_ti(U, 6, 4)
            o02 = view_ti(U, 6, 5)
            q0 = view_ti(sqo, 3, 0)
            q1 = view_ti(sqo, 3, 1)
            q2 = view_ti(sqo, 3, 2)
            det = statp.tile([128, R], DT)
            scr = statp.tile([128, R], DT)
            nc.vector.tensor_mul(det[:], d1, d2)
            nc.vector.tensor_mul(det[:], det[:], d0)
            nc.vector.tensor_mul(scr[:], o01, o12)
            nc.vector.scalar_tensor_tensor(
                scr[:], scr[:], 2.0, o02, op0=Op.mult, op1=Op.mult
            )
            nc.vector.tensor_add(det[:], det[:], scr[:])
            nc.vector.tensor_mul(scr[:], d0, q1)
            nc.vector.tensor_sub(det[:], det[:], scr[:])
            nc.vector.tensor_mul(scr[:], d1, q2)
            nc.vector.tensor_sub(det[:], det[:], scr[:])
            nc.vector.tensor_mul(scr[:], d2, q0)
            nc.vector.tensor_sub(det[:], det[:], scr[:])
            # r = det/2 * rsq^3, clamped
            rec3 = statp.tile([128, R], DT)
            nc.vector.tensor_mul(rec3[:], rsq[:], rsq[:])
            nc.vector.tensor_mul(rec3[:], rec3[:], rsq[:])
            r_t = statp.tile([128, R], DT)
            nc.vector.scalar_tensor_tensor(
                r_t[:], det[:], 0.5, rec3[:], op0=Op.mult, op1=Op.mult
            )
            nc.vector.tensor_scalar(
                r_t[:], r_t[:], 0.999999, -0.999999, op0=Op.min, op1=Op.max
            )
            # f(r): degree-12 polynomial, stt-form Horner
            acc = statp.tile([128, R], DT)
            nc.vector.tensor_scalar(
                acc[:], r_t[:], float(A[10]), float(A[9]), op0=Op.mult, op1=Op.add
            )
            for c in [0.0] + [float(A[k]) for k in range(8, 0, -1)]:
                nc.vector.scalar_tensor_tensor(
                    acc[:], acc[:], c, r_t[:], op0=Op.add, op1=Op.mult
                )
            nc.vector.tensor_scalar_add(acc[:], acc[:], float(A[0]))
            # lam = tr/3 + p * f;  curv = lam / (tr + (K-1)*1e-6)
            pt = statp.tile([128, R], DT)
            nc.vector.tensor_mul(pt[:], p2s[:], rsq[:])
            nc.vector.tensor_mul(pt[:], pt[:], acc[:])
            lam = statp.tile([128, R], DT)
            nc.vector.scalar_tensor_tensor(
                lam[:], tr_t[:], 1.0 / 3.0, pt[:], op0=Op.mult, op1=Op.add
            )
            den = statp.tile([128, R], DT)
            nc.vector.tensor_scalar_add(den[:], tr_t[:], (K - 1) * 1e-6)
            dr = statp.tile([128, R], DT)
            nc.vector.reciprocal(dr[:], den[:])
            curv_all = statp.tile([128, R], DT)
            nc.vector.tensor_mul(curv_all[:], lam[:], dr[:])

            # curv to column order: [128q, 8r] -> [8r, 128q]
            cps = poolB.tile([8, 128], DT, tag="cps")
            nc.tensor.transpose(cps[:], curv_all[:], ident)
            ctv = statp.tile([8, 128], DT)
            nc.vector.tensor_copy(ctv[:], cps[:])
            curv_c = statp.tile([1, G], DT)
            nc.sync.dma_start(
                curv_c.rearrange("o (r q) -> o r q", q=128), ctv[:]
            )

            # strided-partition views for the flo rows (partition 3k+d)
            g42_kd = g42f.rearrange("(k d) c -> d k c", d=D)
            lap_kd = lapT.rearrange("(k d) c -> d k c", d=D)

            poolB.release()
            poolA.release()
            outp = tc.alloc_tile_pool(name="outpsum", bufs=3, space="PSUM")

            # ---- main loop: 4 phases x 8 k x 2 halves ----
            def emit_flo(phase):
                k0 = phase * HK
                flo = flop.tile([6, HK * G], DTR, tag="flo", bufs=2)
                nc.sync.dma_start(
                    flo[0:1, :].rearrange("o (k c) -> o k c", c=G),
                    g42_kd[2:3, k0 : k0 + HK, :].squeeze(0).bitcast(DTR),
                )
                for d in range(D):
                    nc.sync.dma_start(
                        flo[2 + d : 3 + d, :].rearrange("o (k c) -> o k c", c=G),
                        lap_kd[d : d + 1, k0 : k0 + HK, :].squeeze(0).bitcast(DTR),
                    )
                nc.sync.dma_start(
                    flo[5:6, :].rearrange("o (a b) -> o a b", b=HK * G // 128),
                    ones_t.bitcast(DTR),
                )
                nc.sync.dma_start(
                    flo[1:2, :].rearrange("o (k c) -> o k c", c=G),
                    curv_c.bitcast(DTR).unsqueeze(1).broadcast_to([1, HK, G]),
                )
                return flo

            flo = emit_flo(0)
            for phase in range(4):
                k0 = phase * HK
                for k in range(k0, k0 + HK):
                    for half in range(2):
                        csl = slice(half * 512, (half + 1) * 512)
                        xb = xbp.tile([128, 512], DT, tag="xb")
                        nc.tensor.matmul(
                            xb[:],
                            ebig[0:96, k * 128 : (k + 1) * 128],
                            xkdT[:, csl],
                            start=True,
                            stop=True,
                        )
                        # rbf = Exp(-2 * (x - c)^2), c per rbf row
                        t2 = mainp.tile([128, 512], DT, tag="t2")
                        nc.scalar.activation(t2[:], xb[:], Act.Square, bias=negc)
                        fhi = mainp.tile([128, 512], DTR, tag="fhi")
                        nc.scalar.activation(fhi[:], t2[:], Act.Exp, scale=-2.0)
                        so = mainp.tile([128, 4 * OUT], DT, tag="so", bufs=6)
                        so_v = so.rearrange("p (c x) -> p c x", x=OUT)
                        for pair in range(2):
                            ps = outp.tile([128, 1024], DT, tag="ps")
                            for cc in range(2):
                                j = pair * 2 + cc
                                nc.tensor.matmul(
                                    ps[:, cc * 512 : cc * 512 + OUT],
                                    fhi[:, j * 128 : (j + 1) * 128],
                                    whi,
                                    start=True,
                                    stop=False,
                                )
                                lo = (k - k0) * G + half * 512 + j * 128
                                nc.tensor.matmul(
                                    ps[:, cc * 512 : cc * 512 + OUT],
                                    flo[0:6, lo : lo + 128],
                                    wlo,
                                    start=False,
                                    stop=True,
                                )
                            ps_v = ps.rearrange("p (c x) -> p c x", x=512)[:, :, 0:OUT]
                            dst = so_v[:, pair * 2 : pair * 2 + 2, :]
                            cnt = (k * 2 + half) * 2 + pair
                            if cnt % 2 == 1:
                                nc.scalar.copy(dst, ps_v)
                            else:
                                nc.vector.tensor_copy(dst, ps_v)
                        # rows p = g*32 + k = (q*8 + (half*4 + c))*32 + k
                        out_ap = out_d.rearrange(
                            "(q r k) x -> k q r x", r=R, k=K
                        )[k : k + 1, :, half * 4 : (half + 1) * 4, :].squeeze(0)
                        nc.sync.dma_start(
                            out_ap, so.rearrange("p (c x) -> p c x", x=OUT)
                        )
                    # prefetch next phase's flo rows mid-phase so they do not
                    # queue behind this phase's output transfers
                    if k == k0 + 1 and phase < 3:
                        flo_next = emit_flo(phase + 1)
                    if k == k0 + HK - 1 and half == 1 and phase < 3:
                        flo = flo_next
            outp.release()
            xbp.release()

    nc.compile()
    return nc


def _get_program():
    if "nc" not in _prog_cache:
        _prog_cache["nc"] = _build_program()
    return _prog_cache["nc"]


def kernel(xyz, neighbor_xyz, projection_weight, projection_bias):
    from concourse.bass_utils import run_bass_kernel_spmd

    nc = _get_program()

    w = np.ascontiguousarray(projection_weight, dtype=np.float32)
    bias = np.ascontiguousarray(projection_bias, dtype=np.float32)
    whi = np.ascontiguousarray(w[:128])
    wlo6 = np.concatenate([w[128:TOTAL], bias[None, :]], axis=0).astype(np.float32)

    ident = np.eye(128, dtype=np.float32)
    ebase = np.zeros((D, 128), dtype=np.float32)
    for r in range(128):
        ebase[r // F, r] = 1.0
    c = np.linspace(-1.0, 1.0, F + 2, dtype=np.float32)[1:-1]
    negc = (-c[np.arange(128) % F]).reshape(128, 1).astype(np.float32)

    NBA = 128 + 128 + 1
    blob_a = np.zeros((128, NBA), dtype=np.float32)
    blob_a[:, 0:128] = ident
    blob_a[0:D, 128:256] = ebase
    blob_a[:, 256:257] = negc
    blob_b = np.zeros((128, OUT + OUT), dtype=np.float32)
    blob_b[:, 0:OUT] = whi
    blob_b[0:6, OUT : 2 * OUT] = wlo6

    xyz = np.ascontiguousarray(xyz, dtype=np.float32)
    nbr = np.ascontiguousarray(neighbor_xyz, dtype=np.float32)
    in_maps = []
    for core in range(B):
        in_maps.append(
            {
                "xyz": xyz[core].reshape(P, D),
                "nbr": nbr[core].reshape(P, D),
                "blob_a": blob_a,
                "blob_b": blob_b,
            }
        )
    res = run_bass_kernel_spmd(nc, in_maps, list(range(B)))
    out = np.empty((B, S, K, OUT), dtype=np.float32)
    for i in range(B):
        # rows are p = (q*8 + r)*32 + k in natural order already
        out[i] = res.results[i]["out"].reshape(S, K, OUT)
    return out


# revision 21
# speedup vs baseline: 1.2019x; 1.0459x over previous
"""Trainium2 Bass/Tile kernel for EnrichedGeometricEmbedding.

Full-input contract: kernel(**inputs) takes the complete tensors, shards the
batch dim across 8 NeuronCores (B=8 -> 1 batch row per core), runs one SPMD
program via run_bass_kernel_spmd, and gathers the full [8, 1024, 32, 384]
output. Memory-bound problem: the 50 MB/core output write (~140 us at
360 GB/s) sets the roofline; everything else is prologue latency before the
first output DMA can start.

Layout: groups are chunked q-r style (g = q*8 + r, q in [0,128) on the
partition dim) so each input DMA descriptor is a 3 KB contiguous run.  Column
order everywhere downstream is c = r*128 + q.  x is transposed to a
(k,d)-partition layout xkdT [96, 1024] (partition 3k+d) via 8 f32r PE
transposes of [128,96] slices.

Prologue critical path (time-to-curv) is minimized:
  - stats: ACT computes the diagonal second moments (Square), Pool (gpsimd)
    the off-diagonal products, DVE only the mean + one batched reduce; the
    covariance is centered from raw moments (U = P - msum*msum/K).
  - smallest eigenvalue: trig-free.  lam_min = tr/3 + p * f(r) with
    f(r) = 2cos(acos(r)/3 + 2pi/3) evaluated as a degree-12 polynomial
    (max curv abs err 3.4e-3, tolerance ~5e-2), and rsqrt(p2s) via the
    int32 bit-trick + 2 Newton steps - the whole chain lives on DVE, so the
    ACT engine keeps a single function table (exp_and_others) for the whole
    kernel: zero LoadActFuncSet swaps after the initial load.
  - lap rows: y = x - mean subtracted in natural layout on Pool, transposed
    on PE, and the PSUM->SBUF evacuation IS the Abs (ACT reads PSUM).
  - the 2.3 MB one-hot broadcast matrix ebig is built on-chip (memset + 32
    tiny SBUF DMAs from a [3,128] seed) instead of being DMA'd from DRAM.

Main loop (4 phases x 8 k x 2 halves of 512 points) is unchanged in spirit
from the tuned baseline and runs DMA-bound: xb[128,512] = ebig_k^T @ xkdT
broadcasts x to the 128 rbf rows; rbf = Exp(-2*Square(x - c)) on ACT with
per-partition constants; per 128-point tile one K=128 (whi) + one K=6 (wlo)
accumulating f32r matmul into a [128,512] PSUM tile; PSUM->SBUF copies
alternate DVE/ACT; one HWDGE DMA per 512 points scatters [128, 4, 384] rows
to DRAM (1536 B descriptors).  The ones row folds the projection bias into
the matmul.
"""

import numpy as np

B, S, K, D = 8, 1024, 32, 3
F = 43                      # FEAT_DIM
OUT = 384
G = S                       # groups per core
P = S * K                   # points per core (32768)
R = 8                       # groups-per-partition (g = q*8 + r)
TOTAL = F * D + 1 + D       # 133
HK = K // 4                 # k per phase

_prog_cache = {}


def _poly_coeffs():
    """Power-basis coefficients of the deg-12 fit to the lambda_min factor."""
    rr = np.linspace(-1.0, 1.0, 200001)
    f = 2.0 * np.cos(np.arccos(rr) / 3.0 + 2.0 * np.pi / 3.0)
    cheb = np.polynomial.chebyshev.Chebyshev.fit(rr, f, 10)
    return np.polynomial.chebyshev.cheb2poly(cheb.convert().coef)


def _build_program():
    import concourse.bacc as bacc
    import concourse.mybir as mybir
    from concourse.tile import TileContext

    DT = mybir.dt.float32
    DTI = mybir.dt.int32
    DTR = mybir.dt.float32r
    Act = mybir.ActivationFunctionType
    Op = mybir.AluOpType
    X = mybir.AxisListType.X

    C = np.linspace(-1.0, 1.0, F + 2, dtype=np.float64)[1:-1]
    C42 = float(C[F - 1])
    A = _poly_coeffs()          # a[0..12]
    MAGIC = 0x5F3759DF

    nc = bacc.Bacc("TRN2", target_bir_lowering=False, debug=False, num_devices=8)
    xyz_d = nc.dram_tensor("xyz", [P, D], DTR, kind="ExternalInput").ap()
    nbr_d = nc.dram_tensor("nbr", [P, D], DT, kind="ExternalInput").ap()
    # blob_a = ident | E_base | negc   (needed early: transposes + ebig build)
    NBA = 128 + 128 + 1
    # blob_b = whi | wlo6              (needed at the first output matmul)
    NBB = OUT + OUT
    bloba_d = nc.dram_tensor("blob_a", [128, NBA], DTR, kind="ExternalInput").ap()
    blobb_d = nc.dram_tensor("blob_b", [128, NBB], DTR, kind="ExternalInput").ap()
    out_d = nc.dram_tensor("out", [P, OUT], DT, kind="ExternalOutput").ap()

    def view_ti(t24, width, i):
        # column i of a [128, r*width] tile laid out (r, i) -> [128, r]
        return t24.rearrange("p (t i) -> p i t", i=width)[:, i : i + 1, :].squeeze(1)

    with TileContext(nc) as tc:
        with (
            tc.tile_pool(name="const", bufs=1) as constp,
            tc.tile_pool(name="stats", bufs=1) as statp,
            tc.tile_pool(name="gwork", bufs=8) as gwp,
            tc.tile_pool(name="flopool", bufs=1) as flop,
            tc.tile_pool(name="main", bufs=6) as mainp,
        ):
            xbp = tc.alloc_tile_pool(name="xbpsum", bufs=2, space="PSUM")
            poolA = tc.alloc_tile_pool(name="tpsum", bufs=2, space="PSUM")

            # ---- constants built on-chip ----
            ebig = constp.tile([128, K * 128], DTR)
            nc.vector.memset(ebig.bitcast(DT)[:], 0.0)
            ones_t = gwp.tile([128, HK * G // 128], DT, tag="ones", bufs=1)
            nc.vector.memset(ones_t[:], 1.0)

            # ---- input DMAs (sync queue, critical-path order) ----
            n_all = gwp.tile([128, R * K * D], DT, tag="nall", bufs=1)
            nc.sync.dma_start(
                n_all[:], nbr_d.rearrange("(q w) d -> q (w d)", q=128)
            )
            bloba = constp.tile([128, NBA], DTR)
            nc.sync.dma_start(bloba[:], bloba_d[:])
            identR = bloba[:, 0:128]
            ident = identR.bitcast(DT)
            ebase = bloba[0:D, 128:256]
            negc = bloba[:, 256:257].bitcast(DT)
            x_all = gwp.tile([128, R * K * D], DTR, tag="xall", bufs=1)
            nc.sync.dma_start(
                x_all[:], xyz_d.rearrange("(q w) d -> q (w d)", q=128)
            )
            blobb = constp.tile([128, NBB], DTR)
            nc.sync.dma_start(blobb[:], blobb_d[:])
            whi = blobb[:, 0:OUT]
            wlo = blobb[0:6, OUT : 2 * OUT]
            # ebig one-hot diagonal: ebig[3k+d, 128k + r] = 1 iff r//43 == d.
            # Built by log-doubling: seed block 0, then copy the leading
            # [3*2^i, 128*2^i] diagonal square to the next diagonal position.
            nc.sync.dma_start(ebig[0:3, 0:128], ebase)
            for i in range(5):
                w = 1 << i
                nc.sync.dma_start(
                    ebig[3 * w : 6 * w, 128 * w : 256 * w],
                    ebig[0 : 3 * w, 0 : 128 * w],
                )

            # ---- stats: raw moments, mean; products split ACT/Pool ----
            n_v = n_all.rearrange("q (r k d) -> q r d k", k=K, d=D)
            x_v = x_all.bitcast(DT).rearrange("q (r k d) -> q r d k", k=K, d=D)
            prod = gwp.tile([128, R * 6 * K], DT, tag="prod", bufs=1)
            prod_v = prod.rearrange("q (r j k) -> q r j k", j=6, k=K)
            msum = statp.tile([128, R * D], DT)
            msum_v = msum.rearrange("q (r d) -> q r d", d=D)
            # DVE: mean (raw sums; scaling folded into consumers)
            nc.vector.tensor_reduce(msum_v, n_v, axis=X, op=Op.add)
            # ACT: diagonal second moments
            nc.scalar.activation(prod_v[:, :, 0:3, :], n_v, Act.Square)
            # Pool: off-diagonal products
            nc.gpsimd.tensor_mul(
                prod_v[:, :, 3:5, :], n_v[:, :, 0:2, :], n_v[:, :, 1:3, :]
            )
            nc.vector.tensor_mul(
                prod_v[:, :, 5:6, :], n_v[:, :, 0:1, :], n_v[:, :, 2:3, :]
            )
            # Pool: y = x - msum/K  (lap pre-subtraction, natural layout)
            y_all = gwp.tile([128, R * K * D], DTR, tag="yall", bufs=1)
            y_v = y_all.bitcast(DT).rearrange("q (r k d) -> q r d k", k=K, d=D)
            mneg = statp.tile([128, R * D], DT)
            nc.gpsimd.tensor_scalar_mul(mneg[:], msum[:], -1.0 / K)
            y_vr = y_all.rearrange("q (r k d) -> q r d k", k=K, d=D)
            nc.gpsimd.tensor_add(
                y_vr,
                x_v,
                mneg.rearrange("q (r d) -> q r d", d=D)
                .unsqueeze(3)
                .broadcast_to([128, R, D, K]),
            )

            # ---- PE transposes: x then y, 4 r-slices per PSUM bank ----
            xkdT = statp.tile([96, G], DTR)
            lapT = statp.tile([96, G], DT)
            for half in range(2):
                tp = poolA.tile([96, 512], DT, tag="tp")
                for rr in range(4):
                    r = half * 4 + rr
                    nc.tensor.transpose(
                        tp[:, 128 * rr : 128 * (rr + 1)].bitcast(DTR),
                        x_all[:, 96 * r : 96 * (r + 1)],
                        identR,
                    )
                nc.scalar.copy(xkdT[:, 512 * half : 512 * (half + 1)], tp[:])
            for half in range(2):
                tp = poolA.tile([96, 512], DT, tag="tp")
                for rr in range(4):
                    r = half * 4 + rr
                    nc.tensor.transpose(
                        tp[:, 128 * rr : 128 * (rr + 1)].bitcast(DTR),
                        y_all[:, 96 * r : 96 * (r + 1)],
                        identR,
                    )
                # PSUM evacuation doubles as the Abs
                nc.scalar.activation(
                    lapT[:, 512 * half : 512 * (half + 1)], tp[:], Act.Abs
                )

            # ---- g42 row: gaussian of the d=2 plane (pure ACT) ----
            c42b = constp.tile([96, 1], DT)
            nc.vector.memset(c42b[:], -C42)
            g42s = statp.tile([96, G], DT)
            nc.scalar.activation(g42s[:], xkdT.bitcast(DT)[:], Act.Square, bias=c42b[:])
            g42f = statp.tile([96, G], DT)
            nc.scalar.activation(g42f[:], g42s[:], Act.Exp, scale=-2.0)

            # ---- covariance from raw moments ----
            U = statp.tile([128, R * 6], DT)
            U_rj6 = U.rearrange("q (r j) -> q r j", j=6)
            prod_rjk = prod.rearrange("q (r j k) -> q r j k", j=6, k=K)
            nc.vector.tensor_reduce(
                U_rj6[:, :, 3:6], prod_rjk[:, :, 3:6, :], axis=X, op=Op.add
            )
            nc.vector.tensor_reduce(
                U_rj6[:, :, 0:3], prod_rjk[:, :, 0:3, :], axis=X, op=Op.add
            )
            mm = statp.tile([128, R * 6], DT)
            mm_v = mm.rearrange("q (r j) -> q r j", j=6)
            nc.vector.tensor_mul(mm_v[:, :, 0:3], msum_v, msum_v)
            nc.vector.tensor_mul(mm_v[:, :, 3:5], msum_v[:, :, 0:2], msum_v[:, :, 1:3])
            nc.vector.tensor_mul(mm_v[:, :, 5:6], msum_v[:, :, 0:1], msum_v[:, :, 2:3])
            nc.vector.scalar_tensor_tensor(
                U[:], mm[:], -1.0 / K, U[:], op0=Op.mult, op1=Op.add
            )

            # ---- eigen chain, all DVE ----
            U_rj = U.rearrange("q (r j) -> q r j", j=6)
            tr_t = statp.tile([128, R], DT)
            nc.vector.tensor_reduce(tr_t[:], U_rj[:, :, 0:3], axis=X, op=Op.add)
            dd = statp.tile([128, R * 3], DT)
            dd_v = dd.rearrange("q (r i) -> q r i", i=3)
            nc.vector.scalar_tensor_tensor(
                dd_v,
                tr_t[:].unsqueeze(2).broadcast_to([128, R, 3]),
                -1.0 / 3.0,
                U_rj[:, :, 0:3],
                op0=Op.mult,
                op1=Op.add,
            )
            dd2 = statp.tile([128, R * 3], DT)
            nc.vector.tensor_mul(dd2[:], dd[:], dd[:])
            sdd = statp.tile([128, R], DT)
            nc.vector.tensor_reduce(
                sdd[:], dd2.rearrange("q (r i) -> q r i", i=3), axis=X, op=Op.add
            )
            sqo = statp.tile([128, R * 3], DT)
            sqo_v = sqo.rearrange("q (r i) -> q r i", i=3)
            nc.vector.tensor_mul(sqo_v, U_rj[:, :, 3:6], U_rj[:, :, 3:6])
            s2 = statp.tile([128, R], DT)
            nc.vector.tensor_reduce(s2[:], sqo_v, axis=X, op=Op.add)
            p2s = statp.tile([128, R], DT)
            nc.vector.scalar_tensor_tensor(
                p2s[:], s2[:], 2.0, sdd[:], op0=Op.mult, op1=Op.add
            )
            nc.vector.tensor_scalar(
                p2s[:], p2s[:], 1.0 / 6.0, 1e-24, op0=Op.mult, op1=Op.add
            )
            # rsqrt(p2s): int32 bit-trick + 2 Newton steps
            ish = statp.tile([128, R], DT)
            nc.vector.tensor_scalar(
                ish.bitcast(DTI)[:], p2s.bitcast(DTI)[:], 1, None,
                op0=Op.arith_shift_right,
            )
            rsq = statp.tile([128, R], DT)
            nc.vector.tensor_scalar(
                rsq.bitcast(DTI)[:], ish.bitcast(DTI)[:], -1, MAGIC,
                op0=Op.mult, op1=Op.add,
            )
            hlf = statp.tile([128, R], DT)
            nc.vector.tensor_scalar_mul(hlf[:], p2s[:], 0.5)
            nt = statp.tile([128, R], DT)
            for _ in range(1):
                nc.vector.tensor_mul(nt[:], rsq[:], rsq[:])
                nc.vector.tensor_mul(nt[:], nt[:], hlf[:])
                nc.vector.tensor_scalar(
                    nt[:], nt[:], -1.0, 1.5, op0=Op.mult, op1=Op.add
                )
                nc.vector.tensor_mul(rsq[:], rsq[:], nt[:])
            # det of the shifted matrix
            d0 = view_ti(dd, 3, 0)
            d1 = view_ti(dd, 3, 1)
            d2 = view_ti(dd, 3, 2)
            o01 = view_ti(U, 6, 3)
            o12 = view_ti(U, 6, 4)
            o02 = view_ti(U, 6, 5)
            q0 = view_ti(sqo, 3, 0)
            q1 = view_ti(sqo, 3, 1)
            q2 = view_ti(sqo, 3, 2)
            det = statp.tile([128, R], DT)
            scr = statp.tile([128, R], DT)
            nc.gpsimd.tensor_mul(det[:], d1, d2)
            nc.gpsimd.tensor_mul(det[:], det[:], d0)
            nc.gpsimd.tensor_mul(scr[:], o01, o12)
            nc.gpsimd.tensor_mul(scr[:], scr[:], o02)
            nc.gpsimd.tensor_add(det[:], det[:], scr[:])
            nc.gpsimd.tensor_add(det[:], det[:], scr[:])
            nc.gpsimd.tensor_mul(scr[:], d0, q1)
            nc.gpsimd.tensor_sub(det[:], det[:], scr[:])
            nc.gpsimd.tensor_mul(scr[:], d1, q2)
            nc.gpsimd.tensor_sub(det[:], det[:], scr[:])
            nc.gpsimd.tensor_mul(scr[:], d2, q0)
            nc.gpsimd.tensor_sub(det[:], det[:], scr[:])
            # r = det/2 * rsq^3, clamped
            rec3 = statp.tile([128, R], DT)
            nc.vector.tensor_mul(rec3[:], rsq[:], rsq[:])
            nc.vector.tensor_mul(rec3[:], rec3[:], rsq[:])
            r_t = statp.tile([128, R], DT)
            nc.vector.scalar_tensor_tensor(
                r_t[:], det[:], 0.5, rec3[:], op0=Op.mult, op1=Op.mult
            )
            nc.vector.tensor_scalar(
                r_t[:], r_t[:], 0.999999, -0.999999, op0=Op.min, op1=Op.max
            )
            # f(r): degree-12 polynomial, stt-form Horner
            acc = statp.tile([128, R], DT)
            nc.vector.tensor_scalar(
                acc[:], r_t[:], float(A[10]), float(A[9]), op0=Op.mult, op1=Op.add
            )
            for c in [0.0] + [float(A[k]) for k in range(8, 0, -1)]:
                nc.vector.scalar_tensor_tensor(
                    acc[:], acc[:], c, r_t[:], op0=Op.add, op1=Op.mult
                )
            nc.vector.tensor_scalar_add(acc[:], acc[:], float(A[0]))
            # lam = tr/3 + p * f;  curv = lam / (tr + (K-1)*1e-6)
            pt = statp.tile([128, R], DT)
            nc.vector.tensor_mul(pt[:], p2s[:], rsq[:])
            nc.vector.tensor_mul(pt[:], pt[:], acc[:])
            lam = statp.tile([128, R], DT)
            nc.vector.scalar_tensor_tensor(
                lam[:], tr_t[:], 1.0 / 3.0, pt[:], op0=Op.mult, op1=Op.add
            )
            den = statp.tile([128, R], DT)
            nc.vector.tensor_scalar_add(den[:], tr_t[:], (K - 1) * 1e-6)
            dr = statp.tile([128, R], DT)
            nc.vector.reciprocal(dr[:], den[:])
            curv_all = statp.tile([128, R], DT)
            nc.vector.tensor_mul(curv_all[:], lam[:], dr[:])

            # curv to column order: [128q, 8r] -> [8r, 128q] -> [1, (r q)]
            cps = poolA.tile([8, 128], DT, tag="cps")
            nc.tensor.transpose(cps[:], curv_all[:], ident)
            ctv = statp.tile([8, 128], DT)
            nc.vector.tensor_copy(ctv[:], cps[:])
            curv_c = statp.tile([1, G], DT)
            nc.sync.dma_start(
                curv_c.rearrange("o (r q) -> o r q", q=128), ctv[:]
            )

            # strided-partition views for the flo rows (partition 3k+d)
            g42_kd = g42f.rearrange("(k d) c -> d k c", d=D)
            lap_kd = lapT.rearrange("(k d) c -> d k c", d=D)

            poolA.release()
            outp = tc.alloc_tile_pool(name="outpsum", bufs=3, space="PSUM")

            # ---- main loop: 4 phases x 8 k x 2 halves ----
            def emit_flo(phase):
                k0 = phase * HK
                flo = flop.tile([6, HK * G], DTR, tag="flo", bufs=2)
                nc.scalar.dma_start(
                    flo[0:1, :].rearrange("o (k c) -> o k c", c=G),
                    g42_kd[2:3, k0 : k0 + HK, :].squeeze(0).bitcast(DTR),
                )
                for d in range(D):
                    eng = nc.scalar if d == 0 else nc.sync
                    eng.dma_start(
                        flo[2 + d : 3 + d, :].rearrange("o (k c) -> o k c", c=G),
                        lap_kd[d : d + 1, k0 : k0 + HK, :].squeeze(0).bitcast(DTR),
                    )
                nc.sync.dma_start(
                    flo[5:6, :].rearrange("o (a b) -> o a b", b=HK * G // 128),
                    ones_t.bitcast(DTR),
                )
                nc.sync.dma_start(
                    flo[1:2, :].rearrange("o (k c) -> o k c", c=G),
                    curv_c.bitcast(DTR).unsqueeze(1).broadcast_to([1, HK, G]),
                )
                return flo

            flo = emit_flo(0)
            for phase in range(4):
                k0 = phase * HK
                for k in range(k0, k0 + HK):
                    for half in range(2):
                        csl = slice(half * 512, (half + 1) * 512)
                        xb = xbp.tile([128, 512], DT, tag="xb")
                        nc.tensor.matmul(
                            xb[:],
                            ebig[0:96, k * 128 : (k + 1) * 128],
                            xkdT[:, csl],
                            start=True,
                            stop=True,
                        )
                        # rbf = Exp(-2 * (x - c)^2), c per rbf row
                        t2 = mainp.tile([128, 512], DT, tag="t2")
                        nc.scalar.activation(t2[:], xb[:], Act.Square, bias=negc)
                        fhi = mainp.tile([128, 512], DTR, tag="fhi")
                        nc.scalar.activation(fhi[:], t2[:], Act.Exp, scale=-2.0)
                        so = mainp.tile([128, 4 * OUT], DT, tag="so", bufs=6)
                        so_v = so.rearrange("p (c x) -> p c x", x=OUT)
                        for pair in range(2):
                            ps = outp.tile([128, 1024], DT, tag="ps")
                            for cc in range(2):
                                j = pair * 2 + cc
                                nc.tensor.matmul(
                                    ps[:, cc * 512 : cc * 512 + OUT],
                                    fhi[:, j * 128 : (j + 1) * 128],
                                    whi,
                                    start=True,
                                    stop=False,
                                )
                                lo = (k - k0) * G + half * 512 + j * 128
                                nc.tensor.matmul(
                                    ps[:, cc * 512 : cc * 512 + OUT],
                                    flo[0:6, lo : lo + 128],
                                    wlo,
                                    start=False,
                                    stop=True,
                                )
                            ps_v = ps.rearrange("p (c x) -> p c x", x=512)[:, :, 0:OUT]
                            dst = so_v[:, pair * 2 : pair * 2 + 2, :]
                            nc.vector.tensor_copy(dst, ps_v)
                        # rows p = g*32 + k = (q*8 + (half*4 + c))*32 + k
                        out_ap = out_d.rearrange(
                            "(q r k) x -> k q r x", r=R, k=K
                        )[k : k + 1, :, half * 4 : (half + 1) * 4, :].squeeze(0)
                        nc.sync.dma_start(
                            out_ap, so.rearrange("p (c x) -> p c x", x=OUT)
                        )
                    # prefetch next phase's flo rows mid-phase so they do not
                    # queue behind this phase's output transfers
                    if k == k0 + 1 and phase < 3:
                        flo_next = emit_flo(phase + 1)
                    if k == k0 + HK - 1 and half == 1 and phase < 3:
                        flo = flo_next
            outp.release()
            xbp.release()

    nc.compile()
    return nc


def _get_program():
    if "nc" not in _prog_cache:
        _prog_cache["nc"] = _build_program()
    return _prog_cache["nc"]


def kernel(xyz, neighbor_xyz, projection_weight, projection_bias):
    from concourse.bass_utils import run_bass_kernel_spmd

    nc = _get_program()

    w = np.ascontiguousarray(projection_weight, dtype=np.float32)
    bias = np.ascontiguousarray(projection_bias, dtype=np.float32)
    whi = np.ascontiguousarray(w[:128])
    wlo6 = np.concatenate([w[128:TOTAL], bias[None, :]], axis=0).astype(np.float32)

    ident = np.eye(128, dtype=np.float32)
    ebase = np.zeros((D, 128), dtype=np.float32)
    for r in range(128):
        ebase[r // F, r] = 1.0
    c = np.linspace(-1.0, 1.0, F + 2, dtype=np.float32)[1:-1]
    negc = (-c[np.arange(128) % F]).reshape(128, 1).astype(np.float32)

    NBA = 128 + 128 + 1
    blob_a = np.zeros((128, NBA), dtype=np.float32)
    blob_a[:, 0:128] = ident
    blob_a[0:D, 128:256] = ebase
    blob_a[:, 256:257] = negc
    blob_b = np.zeros((128, OUT + OUT), dtype=np.float32)
    blob_b[:, 0:OUT] = whi
    blob_b[0:6, OUT : 2 * OUT] = wlo6

    xyz = np.ascontiguousarray(xyz, dtype=np.float32)
    nbr = np.ascontiguousarray(neighbor_xyz, dtype=np.float32)
    in_maps = []
    for core in range(B):
        in_maps.append(
            {
                "xyz": xyz[core].reshape(P, D),
                "nbr": nbr[core].reshape(P, D),
                "blob_a": blob_a,
                "blob_b": blob_b,
            }
        )
    res = run_bass_kernel_spmd(nc, in_maps, list(range(B)))
    out = np.empty((B, S, K, OUT), dtype=np.float32)
    for i in range(B):
        # rows are p = (q*8 + r)*32 + k in natural order already
        out[i] = res.results[i]["out"].reshape(S, K, OUT)
    return out
